# revision 3
# baseline (speedup 1.0000x reference)
"""Causal self-attention with RoPE on 8 Trainium2 NeuronCores — v4.

Full inputs: x [4, 2048, 1024], W_attn [1024, 3072], W_proj [1024, 1024] (f32).
Sharding: core = b*2 + hg  (4 batches x 2 head-groups of 8 heads).
Host sums the two head-group partials per batch.

v4 design (all-bf16 data path):
- qk GEMM bf16 (stationary W tiles, moving xT), v GEMM bf16.
- RoPE: one PSUM->SBUF copy, then 4x-mode bf16 shuffle/mul/add on DVE.
- S = K^T Q per (head, 128-key tile) in bf16; causal masking by accumulating
  an additive -30000 mask block into PSUM via a second matmul (identity
  stationary), so exp() of masked entries is exactly 0.
- exp: batched ACT instructions over a [128, 1024] PSUM pair-arena with the
  diagonal blocks packed at exact widths (no wasted exp columns).
- AV transposed: stationary = expT tile [128k, 128q], moving = v [128k, 65]
  (65th column of v is ones -> denominator lands in output col 64).
- y finalize: copy/recip/scale (DVE) then PE transpose back to [d, t] layout.
- Output projection bf16, output DMA'd as bf16 and upcast on host.
"""

import sys

sys.path.insert(0, "/opt/trn_rl_repo")

import numpy as np

import concourse.bass as bass  # noqa: F401
import concourse.mybir as mybir
import concourse.tile as tile
from concourse import bacc
from concourse.bass_utils import run_bass_kernel_spmd

F32 = mybir.dt.float32
BF16 = mybir.dt.bfloat16
AF = mybir.ActivationFunctionType
OP = mybir.AluOpType

B, T, C = 4, 2048, 1024
H, D = 16, 64
HPC = 8                 # heads per core
N_KO = C // 128         # 8 contraction chunks
TC = 512                # t-chunk width in phase A
N_TC = T // TC          # 4
N_TT = T // 128         # 16 key tiles
N_IC = 4                # i-chunks of 512 queries
IC = 512
NEG = -30000.0

# diag pack: widths of the 4 diagonal jt blocks and their column offsets in
# the two diag psum tiles (A: [512|384] -> 896 cols, B: [256|128] -> 384)
DIAG_W = [512, 384, 256, 128]
DIAG_TILE = [0, 0, 1, 1]        # which diag tile
DIAG_OFF = [0, 512, 0, 256]     # col offset within its tile
DIAG_TW = [896, 384]            # widths of diag tiles A and B


def _rope_tables():
    """cosT/sinN [128, T]: row p holds tables for d = p % 64; sinN has the
    rotate-half sign folded in (rows with d%64 < 32 negative)."""
    inv_freq = (
        np.float32(1.0)
        / np.float32(10000.0) ** (np.arange(0, D, 2, dtype=np.float32) / np.float32(D))
    ).astype(np.float32)
    t = np.arange(T, dtype=np.float32)
    freqs = (t[:, None] * inv_freq[None, :]).astype(np.float32)  # [T, 32]
    emb = np.concatenate([freqs, freqs], axis=1)  # [T, 64]
    cos = np.cos(emb).astype(np.float32)
    sin = np.sin(emb).astype(np.float32)
    sinN = np.concatenate([-sin[:, :32], sin[:, 32:]], axis=1)
    cosT = np.tile(cos.T, (2, 1))   # [128, T]
    sinNT = np.tile(sinN.T, (2, 1))
    return np.ascontiguousarray(cosT), np.ascontiguousarray(sinNT)


def _mask_table():
    """maskT [128, 128]: 0.0 if k <= q else NEG  (S^T layout: [keys, queries])."""
    k = np.arange(128)[:, None]
    q = np.arange(128)[None, :]
    return np.where(k <= q, 0.0, NEG).astype(np.float32)


def _build():
    nc = bacc.Bacc(None, target_bir_lowering=False, debug=False)

    xT = nc.dram_tensor("xT", [C, T], BF16, kind="ExternalInput")
    wqk = nc.dram_tensor("wqk", [C, 1024], BF16, kind="ExternalInput")
    wv = nc.dram_tensor("wv", [C, 512], BF16, kind="ExternalInput")
    wproj = nc.dram_tensor("wproj", [HPC * D, C], BF16, kind="ExternalInput")
    cosT_d = nc.dram_tensor("cosT", [128, T], BF16, kind="ExternalInput")
    sinN_d = nc.dram_tensor("sinN", [128, T], BF16, kind="ExternalInput")
    mask_d = nc.dram_tensor("maskT", [128, 128], BF16, kind="ExternalInput")
    id_d = nc.dram_tensor("ident", [128, 128], BF16, kind="ExternalInput")
    out_d = nc.dram_tensor("out", [T, C], BF16, kind="ExternalOutput")

    xT_r = xT.rearrange("(ko p) t -> p ko t", p=128)
    wqk_r = wqk.rearrange("(ko p) c -> p ko c", p=128)
    wv_r = wv.rearrange("(ko p) c -> p ko c", p=128)
    wproj_r = wproj.rearrange("(hp p) c -> p hp c", p=128)

    with tile.TileContext(nc) as tc:
        with (
            tc.tile_pool(name="res", bufs=1) as res,
            tc.tile_pool(name="qkv", bufs=1) as qkvp,
            tc.tile_pool(name="xt", bufs=2) as xtp,
            tc.tile_pool(name="rope", bufs=4) as ropep,
            tc.tile_pool(name="exp", bufs=4) as expp,
            tc.tile_pool(name="fin", bufs=2) as finp,
            tc.tile_pool(name="ost", bufs=3) as ostp,
            tc.tile_pool(name="ps_a", bufs=2, space="PSUM") as ps_a,
            tc.tile_pool(name="ps_s", bufs=2, space="PSUM") as ps_s,
            tc.tile_pool(name="ps_y", bufs=1, space="PSUM") as ps_y,
            tc.tile_pool(name="ps_o", bufs=1, space="PSUM") as ps_o,
        ):
            # ---- resident tables / weights ----
            cos_sb = res.tile([128, T], BF16)
            sinN_sb = res.tile([128, T], BF16)
            mask_sb = res.tile([128, 128], BF16)
            id_sb = res.tile([128, 128], BF16)
            wqk_sb = res.tile([128, N_KO, 1024], BF16)
            wv_sb = res.tile([128, N_KO, 512], BF16)
            wp_sb = res.tile([128, 4, C], BF16)

            # ---- phase A outputs ----
            qT = qkvp.tile([128, 4, T], BF16)   # p = hl*64+d, head 2hp+hl
            kT = qkvp.tile([128, 4, T], BF16)
            v_sb = qkvp.tile([128, N_TT, HPC, D + 1], BF16)
            yT_sb = qkvp.tile([128, 4, T], BF16)

            # first-needed-first DMA order: xt chunk 0 interleaved with wqk,
            # then tables, then wv/wproj
            xt0 = xtp.tile([128, N_KO, TC], BF16, name="xt0", tag="xt")
            for ko in range(N_KO):
                nc.sync.dma_start(xt0[:, ko], xT_r[:, ko, 0:TC])
                nc.sync.dma_start(wqk_sb[:, ko], wqk_r[:, ko])
            nc.sync.dma_start(cos_sb[:], cosT_d[:])
            nc.sync.dma_start(sinN_sb[:], sinN_d[:])
            nc.sync.dma_start(mask_sb[:], mask_d[:])
            nc.sync.dma_start(id_sb[:], id_d[:])
            for ko in range(N_KO):
                nc.sync.dma_start(wv_sb[:, ko], wv_r[:, ko])
            nc.sync.dma_start(wp_sb[:], wproj_r[:])

            # ones columns of v (written once)
            nc.gpsimd.memset(v_sb[:, :, :, D], 1.0)

            for cch in range(N_TC):
                _phase_a_chunk(
                    nc, cch, xT_r, xt0 if cch == 0 else None,
                    xtp, ropep, ps_a, wqk_sb, wv_sb, cos_sb, sinN_sb,
                    qT, kT, v_sb,
                )
                _attention_ic(
                    nc, cch, qT, kT, v_sb, yT_sb, mask_sb, id_sb,
                    expp, finp, ps_s, ps_y, ps_o, [],
                )
                for f in _proj_units(nc, cch, yT_sb, wp_sb, ostp, ps_o, out_d):
                    f()

    nc.compile()
    return nc


def _phase_a_units(nc, cch, xT_r, xtp, ropep, ps_a, wqk_sb, wv_sb,
                   cos_sb, sinN_sb, qT, kT, v_sb, xt0=None):
    """Return a list of closures, each emitting one qk column tile (GEMM +
    rope) or one v sub-tile for t-chunk cch."""
    ts_ = slice(cch * TC, (cch + 1) * TC)
    state = {}

    def get_xt():
        if "xt" not in state:
            if xt0 is not None:
                state["xt"] = xt0
            else:
                xt_sb = xtp.tile([128, N_KO, TC], BF16, name="xt", tag="xt")
                for ko in range(N_KO):
                    nc.sync.dma_start(xt_sb[:, ko], xT_r[:, ko, ts_])
                state["xt"] = xt_sb
        return state["xt"]

    def qk_unit(ct):
        def emit():
            xt_sb = get_xt()
            psum = ps_a.tile([128, TC], F32, name="ps_qk", tag="ps_a")
            for ko in range(N_KO):
                nc.tensor.matmul(
                    psum[:],
                    wqk_sb[:, ko, ct * 128:(ct + 1) * 128],
                    xt_sb[:, ko, :],
                    start=(ko == 0),
                    stop=(ko == N_KO - 1),
                )
            hp = ct % 4
            dest = (qT if ct < 4 else kT)[:, hp, ts_]
            # rope: raw copy out of PSUM, then 4x bf16 shuffle + mul + add
            raw = ropep.tile([128, TC], BF16, name="raw", tag="raw")
            with nc.allow_low_precision(reason="rope bf16"):
                nc.scalar.copy(raw[:], psum[:])
            rot = ropep.tile([128, TC], BF16, name="rot", tag="rot")
            for blk in range(4):
                src = (blk ^ 1) * 32
                nc.vector.tensor_copy(
                    rot[blk * 32: blk * 32 + 32, :], raw[src: src + 32, :]
                )
            t1 = ropep.tile([128, TC], BF16, name="t1", tag="t1")
            with nc.allow_low_precision(reason="rope bf16"):
                nc.vector.tensor_tensor(t1[:], raw[:], cos_sb[:, ts_], OP.mult)
                nc.vector.tensor_tensor(rot[:], rot[:], sinN_sb[:, ts_], OP.mult)
                nc.vector.tensor_tensor(dest, t1[:], rot[:], OP.add)
        return emit

    def v_unit(sub):
        def emit():
            xt_sb = get_xt()
            tt = cch * (TC // 128) + sub
            psv = ps_a.tile([128, HPC * D], F32, name="ps_v", tag="ps_a")
            for ko in range(N_KO):
                nc.tensor.matmul(
                    psv[:],
                    xt_sb[:, ko, sub * 128: sub * 128 + 128],
                    wv_sb[:, ko, :],
                    start=(ko == 0),
                    stop=(ko == N_KO - 1),
                )
            with nc.allow_low_precision(reason="v bf16"):
                nc.scalar.copy(
                    v_sb[:, tt, :, 0:D],
                    psv[:].rearrange("p (h d) -> p h d", d=D),
                )
        return emit

    # k and v tiles first: attention(cch) itself only needs q of chunk cch,
    # but attention(cch) woven around these units consumes k/v of chunk cch
    # only, so order within the chunk is free; keep qk first for rope flow.
    units = [qk_unit(ct) for ct in range(8)]
    units += [v_unit(sub) for sub in range(TC // 128)]
    return units


def _phase_a_chunk(nc, cch, xT_r, xt0, xtp, ropep, ps_a, wqk_sb, wv_sb,
                   cos_sb, sinN_sb, qT, kT, v_sb):
    for f in _phase_a_units(nc, cch, xT_r, xtp, ropep, ps_a, wqk_sb, wv_sb,
                            cos_sb, sinN_sb, qT, kT, v_sb, xt0=xt0):
        f()


def _attention_ic(nc, ic, qT, kT, v_sb, yT_sb, mask_sb, id_sb,
                  expp, finp, ps_s, ps_y, ps_o, filler):
    """Attention for query chunk ic (512 queries), all 4 head pairs.
    Pops one PE filler unit per (hp, hl) sub-loop to keep PE fed while
    ACT drains exp."""
    n_jt = (ic + 1) * 4
    is_ = slice(ic * IC, (ic + 1) * IC)
    # Bresenham-spread filler over the exp groups of this ic
    n_groups = 8 * (2 * ic + 2)
    n_fill = len(filler)
    gidx = [0]

    def pop_filler():
        want = ((gidx[0] + 1) * n_fill) // n_groups
        done = (gidx[0] * n_fill) // n_groups
        for _ in range(want - done):
            if filler:
                filler.pop(0)()
        gidx[0] += 1

    for hp in range(4):
        ytr = ps_o.tile([128, IC], BF16, name="ytr", tag="ps_o")
        for hl in range(2):
            pb = hl * 64
            h = 2 * hp + hl
            ypsum = ps_y.tile([128, 4, D + 1], F32, name="ypsum", tag="ypsum")
            # emit S+exp+AV jt-group-major so ACT drains while PE fills
            groups = []  # (psum_width, [(jt, off, w, masked)])
            for j0 in range(0, 4 * ic, 2):
                groups.append(
                    (1024, [(j0, 0, 512, False), (j0 + 1, 512, 512, False)])
                )
            groups.append(
                (1024, [(4 * ic, 0, 512, True), (4 * ic + 1, 512, 384, True)])
            )
            groups.append(
                (512, [(4 * ic + 2, 0, 256, True), (4 * ic + 3, 256, 128, True)])
            )
            def emit_s_exp(width, blocks):
                sp = ps_s.tile([128, 1024], F32, name="sp", tag="sp")
                for jt, off, w, masked in blocks:
                    lo = max(0, (jt - 4 * ic)) * 128
                    # start=True zeroes the whole 2KB PSUM bank: only the
                    # first block in each bank may set it
                    nc.tensor.matmul(
                        sp[:, off: off + w],
                        kT[pb: pb + 64, hp, jt * 128:(jt + 1) * 128],
                        qT[pb: pb + 64, hp, ic * IC + lo:(ic + 1) * IC],
                        start=(off % 512 == 0),
                        stop=not masked,
                        skip_group_check=True,
                    )
                    if masked:
                        nc.tensor.matmul(
                            sp[:, off: off + 128],
                            id_sb[:],
                            mask_sb[:],
                            start=False,
                            stop=True,
                            skip_group_check=True,
                        )
                ew = sum(b[2] for b in blocks)
                et = expp.tile([128, 1024], BF16, name="et", tag="et")
                with nc.allow_low_precision(reason="exp bf16"):
                    nc.scalar.activation(et[:, 0:ew], sp[:, 0:ew], AF.Exp)
                return et

            def emit_av(et, blocks):
                for jt, off, w, masked in blocks:
                    lo = max(0, (jt - 4 * ic)) * 128
                    for qs in range(4):
                        if qs * 128 < lo:
                            continue
                        col0 = off + qs * 128 - lo
                        nc.tensor.matmul(
                            ypsum[:, qs, :],
                            et[:, col0: col0 + 128],
                            v_sb[:, jt, h, :],
                            start=(jt == 0 and qs == 0),
                            stop=(jt == 4 * ic + qs),
                            skip_group_check=True,
                        )

            for width, blocks in groups:
                et = emit_s_exp(width, blocks)
                emit_av(et, blocks)
                pop_filler()
            # finalize: copy, recip, scale, transpose into ytr bank
            ysb = finp.tile([128, 4, D + 1], BF16, name="ysb", tag="ysb")
            nc.vector.tensor_copy(ysb[:], ypsum[:])
            rsb = finp.tile([128, 4], F32, name="rsb", tag="rsb")
            with nc.allow_low_precision(reason="softmax recip"):
                nc.vector.reciprocal(rsb[:], ysb[:, :, D])
            yn = finp.tile([128, 4, D], BF16, name="yn", tag="yn")
            with nc.allow_low_precision(reason="softmax scale bf16"):
                for qs in range(4):
                    nc.vector.tensor_scalar_mul(
                        yn[:, qs, :], ysb[:, qs, 0:D], rsb[:, qs: qs + 1]
                    )
            for qs in range(4):
                nc.tensor.matmul(
                    ytr[pb: pb + 64, qs * 128:(qs + 1) * 128],
                    yn[:, qs, :],
                    id_sb[:],
                    is_transpose=True,
                    start=(hl == 0 and qs == 0),
                    stop=(hl == 1 and qs == 3),
                    skip_group_check=True,
                )
        nc.vector.tensor_copy(yT_sb[:, hp, is_], ytr[:])


def _proj_units(nc, ic, yT_sb, wp_sb, ostp, ps_o, out_d):
    def unit(tt, cc):
        def emit():
            po = ps_o.tile([128, 512], F32, name="po", tag="ps_o")
            for hp in range(4):
                nc.tensor.matmul(
                    po[:],
                    yT_sb[:, hp, tt * 128:(tt + 1) * 128],
                    wp_sb[:, hp, cc * 512:(cc + 1) * 512],
                    start=(hp == 0),
                    stop=(hp == 3),
                )
            ost = ostp.tile([128, 512], BF16)
            with nc.allow_low_precision(reason="bf16 output"):
                nc.scalar.copy(ost[:], po[:])
            nc.sync.dma_start(
                out_d[tt * 128:(tt + 1) * 128, cc * 512:(cc + 1) * 512],
                ost[:],
            )
        return emit

    return [unit(tt, cc)
            for tt in range(ic * 4, (ic + 1) * 4) for cc in range(2)]


_NC = None
_STATE = None


def _get_nc():
    global _NC
    if _NC is None:
        _NC = _build()
    return _NC


def _tables_np():
    import ml_dtypes
    bf = ml_dtypes.bfloat16
    cosT, sinN = _rope_tables()
    return {
        "cosT": cosT.astype(bf),
        "sinN": sinN.astype(bf),
        "maskT": _mask_table().astype(bf),
        "ident": np.eye(128, dtype=np.float32).astype(bf),
    }


def _prep_w(W_attn, W_proj):
    """Per-head-group weight slices, bf16, quad-dedup'd (each core sends a
    quarter of its head-group's weights)."""
    import ml_dtypes
    bf = ml_dtypes.bfloat16
    scale = np.float32(1.0 / np.sqrt(D))
    wqk_hg, wv_hg, wp_hg = [], [], []
    for hg in range(2):
        cs = slice(hg * HPC * D, (hg + 1) * HPC * D)
        wq = W_attn[:, 0 * C:][:, cs] * scale
        wk = W_attn[:, 1 * C:][:, cs]
        wv = W_attn[:, 2 * C:][:, cs]
        wqk_hg.append(np.concatenate([wq, wk], axis=1).astype(bf))
        wv_hg.append(np.ascontiguousarray(wv).astype(bf))
        wp_hg.append(np.ascontiguousarray(W_proj[cs, :]).astype(bf))
    wqkc = np.empty((8 * 128, 1024), dtype=bf)
    wvc = np.empty((8 * 128, 512), dtype=bf)
    wpc = np.empty((8 * 64, C), dtype=bf)
    for c in range(8):
        q = c // 2
        wqkc[c * 128:(c + 1) * 128] = wqk_hg[c % 2][q * 128:(q + 1) * 128]
        wvc[c * 128:(c + 1) * 128] = wv_hg[c % 2][q * 128:(q + 1) * 128]
        wpc[c * 64:(c + 1) * 64] = wp_hg[c % 2][q * 64:(q + 1) * 64]
    return wqkc, wvc, wpc


def _prep_compact(x, W_attn, W_proj):
    import ml_dtypes
    xs = np.ascontiguousarray(x.reshape(8 * (T // 2), C)).astype(
        ml_dtypes.bfloat16
    )
    wqkc, wvc, wpc = _prep_w(W_attn, W_proj)
    return xs, wqkc, wvc, wpc


def _get_state():
    global _STATE
    if _STATE is not None:
        return _STATE

    import jax
    import jax.numpy as jnp
    from jax.experimental.shard_map import shard_map
    from jax.sharding import Mesh, NamedSharding, PartitionSpec

    from concourse import bass2jax

    nc = _get_nc()
    bass2jax.install_neuronx_cc_hook()
    partition_name = nc.partition_id_tensor.name if nc.partition_id_tensor else None
    in_names, out_names, out_avals = [], [], []
    for alloc in nc.m.functions[0].allocations:
        if not isinstance(alloc, mybir.MemoryLocationSet):
            continue
        name = alloc.memorylocations[0].name
        if alloc.kind == "ExternalInput":
            if name != partition_name:
                in_names.append(name)
        elif alloc.kind == "ExternalOutput":
            out_names.append(name)
            out_avals.append(
                jax.core.ShapedArray(tuple(alloc.tensor_shape), mybir.dt.np(alloc.dtype))
            )
    n_params, n_outs = len(in_names), len(out_avals)
    all_names = list(in_names) + out_names
    if partition_name:
        all_names.append(partition_name)

    def _body(*args):
        operands = list(args)
        if partition_name:
            operands.append(bass2jax.partition_id_tensor())
        outs = bass2jax._bass_exec_p.bind(
            *operands,
            out_avals=tuple(out_avals),
            in_names=tuple(all_names),
            out_names=tuple(out_names),
            lowering_input_output_aliases=(),
            sim_require_finite=True,
            sim_require_nnan=True,
            nc=nc,
        )
        return tuple(outs)

    devices = jax.devices()[:8]
    mesh = Mesh(np.asarray(devices), ("core",))
    shd = NamedSharding(mesh, PartitionSpec("core"))
    donate = tuple(range(n_params, n_params + n_outs))
    sharded = jax.jit(
        shard_map(
            _body,
            mesh=mesh,
            in_specs=(PartitionSpec("core"),) * (n_params + n_outs),
            out_specs=(PartitionSpec("core"),) * n_outs,
            check_rep=False,
        ),
        donate_argnums=donate,
        keep_unused=True,
    )
    zeros_fn = jax.jit(
        lambda: tuple(
            jnp.zeros((8 * av.shape[0],) + av.shape[1:], av.dtype) for av in out_avals
        ),
        out_shardings=(shd,) * n_outs,
    )

    tabs = _tables_np()
    statics = {
        k: jax.device_put(np.tile(v, (8, 1)), shd) for k, v in tabs.items()
    }
    jax.block_until_ready(list(statics.values()))

    PAIRS = [[0, 1], [2, 3], [4, 5], [6, 7]]
    QUADS = [[0, 2, 4, 6], [1, 3, 5, 7]]

    def _pre(xs, wqk, wv, wp):
        xg = jax.lax.all_gather(xs, "core", axis_index_groups=PAIRS, axis=0, tiled=True)
        wqkg = jax.lax.all_gather(wqk, "core", axis_index_groups=QUADS, axis=0, tiled=True)
        wvg = jax.lax.all_gather(wv, "core", axis_index_groups=QUADS, axis=0, tiled=True)
        wpg = jax.lax.all_gather(wp, "core", axis_index_groups=QUADS, axis=0, tiled=True)
        zeros = tuple(jnp.zeros(av.shape, av.dtype) for av in out_avals)
        return (xg.T, wqkg, wvg, wpg) + zeros

    pre_fn = jax.jit(
        shard_map(
            _pre,
            mesh=mesh,
            in_specs=(PartitionSpec("core"),) * 4,
            out_specs=(PartitionSpec("core"),) * (4 + n_outs),
        )
    )

    def _post(o):
        other = jax.lax.ppermute(
            o, "core", [(0, 1), (1, 0), (2, 3), (3, 2), (4, 5), (5, 4), (6, 7), (7, 6)]
        )
        s = o.astype(jnp.float32) + other.astype(jnp.float32)
        idx = jax.lax.axis_index("core")
        return jax.lax.dynamic_slice(s, ((idx % 2) * (T // 2), 0), (T // 2, C))

    post_fn = jax.jit(
        shard_map(
            _post,
            mesh=mesh,
            in_specs=(PartitionSpec("core"),),
            out_specs=PartitionSpec("core"),
        )
    )

    _STATE = dict(
        jax=jax,
        nc=nc,
        in_names=in_names,
        out_names=out_names,
        n_outs=n_outs,
        sharded=sharded,
        zeros_fn=zeros_fn,
        shd=shd,
        statics=statics,
        pre_fn=pre_fn,
        post_fn=post_fn,
    )
    return _STATE


def _run_gathered(st, x, W_attn, W_proj):
    jax = st["jax"]
    import ml_dtypes
    xs = np.ascontiguousarray(x.reshape(8 * (T // 2), C)).astype(
        ml_dtypes.bfloat16
    )
    d_xs = jax.device_put(xs, st["shd"])
    wqkc, wvc, wpc = _prep_w(W_attn, W_proj)
    d_wqk = jax.device_put(wqkc, st["shd"])
    d_wv = jax.device_put(wvc, st["shd"])
    d_wp = jax.device_put(wpc, st["shd"])
    pre = st["pre_fn"](d_xs, d_wqk, d_wv, d_wp)
    dyn = {"xT": pre[0], "wqk": pre[1], "wv": pre[2], "wproj": pre[3]}
    args = [dyn[nm] if nm in dyn else st["statics"][nm] for nm in st["in_names"]]
    outs = st["sharded"](*args, *pre[4:])
    po = st["post_fn"](outs[0])
    r = np.asarray(po).reshape(B, T, C)
    return np.ascontiguousarray(r)


def _run_rbks(x, W_attn, W_proj):
    """Fallback: the stock run_bass_kernel_spmd entry point."""
    import ml_dtypes
    bf = ml_dtypes.bfloat16
    nc = _get_nc()
    tabs = _tables_np()
    scale = np.float32(1.0 / np.sqrt(D))
    in_maps = []
    for core in range(8):
        b, hg = core // 2, core % 2
        cs = slice(hg * HPC * D, (hg + 1) * HPC * D)
        wq = W_attn[:, 0 * C:][:, cs] * scale
        wk = W_attn[:, 1 * C:][:, cs]
        wv = W_attn[:, 2 * C:][:, cs]
        m = {
            "xT": np.ascontiguousarray(x[b].T).astype(bf),
            "wqk": np.concatenate([wq, wk], axis=1).astype(bf),
            "wv": np.ascontiguousarray(wv).astype(bf),
            "wproj": np.ascontiguousarray(W_proj[cs, :]).astype(bf),
        }
        m.update(tabs)
        in_maps.append(m)
    res = run_bass_kernel_spmd(nc, in_maps, core_ids=list(range(8)))
    out = np.empty((B, T, C), dtype=np.float32)
    for b in range(B):
        out[b] = res.results[2 * b]["out"].astype(np.float32) + res.results[
            2 * b + 1
        ]["out"].astype(np.float32)
    return out


def kernel(x, W_attn, W_proj):
    x = np.asarray(x, dtype=np.float32)
    W_attn = np.asarray(W_attn, dtype=np.float32)
    W_proj = np.asarray(W_proj, dtype=np.float32)

    try:
        st = _get_state()
        return _run_gathered(st, x, W_attn, W_proj)
    except Exception:
        return _run_rbks(x, W_attn, W_proj)


if __name__ == "__main__":
    nc = _get_nc()
    from concourse.timeline_sim import TimelineSim
    sim_ns = TimelineSim(nc, trace=False).simulate()
    print(f"timeline-sim: {sim_ns/1e3:.1f} us")


# revision 4
# speedup vs baseline: 1.0203x; 1.0203x over previous
"""Causal self-attention with RoPE on 8 Trainium2 NeuronCores — v4.

Full inputs: x [4, 2048, 1024], W_attn [1024, 3072], W_proj [1024, 1024] (f32).
Sharding: core = b*2 + hg  (4 batches x 2 head-groups of 8 heads).
Host sums the two head-group partials per batch.

v4 design (all-bf16 data path):
- qk GEMM bf16 (stationary W tiles, moving xT), v GEMM bf16.
- RoPE: one PSUM->SBUF copy, then 4x-mode bf16 shuffle/mul/add on DVE.
- S = K^T Q per (head, 128-key tile) in bf16; causal masking by accumulating
  an additive -30000 mask block into PSUM via a second matmul (identity
  stationary), so exp() of masked entries is exactly 0.
- exp: batched ACT instructions over a [128, 1024] PSUM pair-arena with the
  diagonal blocks packed at exact widths (no wasted exp columns).
- AV transposed: stationary = expT tile [128k, 128q], moving = v [128k, 65]
  (65th column of v is ones -> denominator lands in output col 64).
- y finalize: copy/recip/scale (DVE) then PE transpose back to [d, t] layout.
- Output projection bf16, output DMA'd as bf16 and upcast on host.
"""

import sys

sys.path.insert(0, "/opt/trn_rl_repo")

import numpy as np

import concourse.bass as bass  # noqa: F401
import concourse.mybir as mybir
import concourse.tile as tile
from concourse import bacc
from concourse.bass_utils import run_bass_kernel_spmd

F32 = mybir.dt.float32
BF16 = mybir.dt.bfloat16
AF = mybir.ActivationFunctionType
OP = mybir.AluOpType

B, T, C = 4, 2048, 1024
H, D = 16, 64
HPC = 8                 # heads per core
N_KO = C // 128         # 8 contraction chunks
TC = 512                # t-chunk width in phase A
N_TC = T // TC          # 4
N_TT = T // 128         # 16 key tiles
N_IC = 4                # i-chunks of 512 queries
IC = 512
NEG = -30000.0

# diag pack: widths of the 4 diagonal jt blocks and their column offsets in
# the two diag psum tiles (A: [512|384] -> 896 cols, B: [256|128] -> 384)
DIAG_W = [512, 384, 256, 128]
DIAG_TILE = [0, 0, 1, 1]        # which diag tile
DIAG_OFF = [0, 512, 0, 256]     # col offset within its tile
DIAG_TW = [896, 384]            # widths of diag tiles A and B


def _rope_tables():
    """cosT/sinN [128, T]: row p holds tables for d = p % 64; sinN has the
    rotate-half sign folded in (rows with d%64 < 32 negative)."""
    inv_freq = (
        np.float32(1.0)
        / np.float32(10000.0) ** (np.arange(0, D, 2, dtype=np.float32) / np.float32(D))
    ).astype(np.float32)
    t = np.arange(T, dtype=np.float32)
    freqs = (t[:, None] * inv_freq[None, :]).astype(np.float32)  # [T, 32]
    emb = np.concatenate([freqs, freqs], axis=1)  # [T, 64]
    cos = np.cos(emb).astype(np.float32)
    sin = np.sin(emb).astype(np.float32)
    sinN = np.concatenate([-sin[:, :32], sin[:, 32:]], axis=1)
    cosT = np.tile(cos.T, (2, 1))   # [128, T]
    sinNT = np.tile(sinN.T, (2, 1))
    return np.ascontiguousarray(cosT), np.ascontiguousarray(sinNT)


def _mask_table():
    """maskT [128, 128]: 0.0 if k <= q else NEG  (S^T layout: [keys, queries])."""
    k = np.arange(128)[:, None]
    q = np.arange(128)[None, :]
    return np.where(k <= q, 0.0, NEG).astype(np.float32)


def _build():
    nc = bacc.Bacc(None, target_bir_lowering=False, debug=False)

    xT = nc.dram_tensor("xT", [C, T], BF16, kind="ExternalInput")
    wqk = nc.dram_tensor("wqk", [C, 1024], BF16, kind="ExternalInput")
    wv = nc.dram_tensor("wv", [C, 512], BF16, kind="ExternalInput")
    wproj = nc.dram_tensor("wproj", [HPC * D, C], BF16, kind="ExternalInput")
    cosT_d = nc.dram_tensor("cosT", [128, T], BF16, kind="ExternalInput")
    sinN_d = nc.dram_tensor("sinN", [128, T], BF16, kind="ExternalInput")
    mask_d = nc.dram_tensor("maskT", [128, 128], BF16, kind="ExternalInput")
    id_d = nc.dram_tensor("ident", [128, 128], BF16, kind="ExternalInput")
    out_d = nc.dram_tensor("out", [T, C], BF16, kind="ExternalOutput")

    xT_r = xT.rearrange("(ko p) t -> p ko t", p=128)
    wqk_r = wqk.rearrange("(ko p) c -> p ko c", p=128)
    wv_r = wv.rearrange("(ko p) c -> p ko c", p=128)
    wproj_r = wproj.rearrange("(hp p) c -> p hp c", p=128)

    with tile.TileContext(nc) as tc:
        with (
            tc.tile_pool(name="res", bufs=1) as res,
            tc.tile_pool(name="qkv", bufs=1) as qkvp,
            tc.tile_pool(name="xt", bufs=2) as xtp,
            tc.tile_pool(name="rope", bufs=4) as ropep,
            tc.tile_pool(name="exp", bufs=4) as expp,
            tc.tile_pool(name="fin", bufs=3) as finp,
            tc.tile_pool(name="ost", bufs=4) as ostp,
            tc.tile_pool(name="ps_a", bufs=2, space="PSUM") as ps_a,
            tc.tile_pool(name="ps_s", bufs=2, space="PSUM") as ps_s,
            tc.tile_pool(name="ps_y", bufs=1, space="PSUM") as ps_y,
            tc.tile_pool(name="ps_o", bufs=1, space="PSUM") as ps_o,
        ):
            # ---- resident tables / weights ----
            cos_sb = res.tile([128, T], BF16)
            sinN_sb = res.tile([128, T], BF16)
            mask_sb = res.tile([128, 128], BF16)
            id_sb = res.tile([128, 128], BF16)
            wqk_sb = res.tile([128, N_KO, 1024], BF16)
            wv_sb = res.tile([128, N_KO, 512], BF16)
            wp_sb = res.tile([128, 4, C], BF16)

            # ---- phase A outputs ----
            qT = qkvp.tile([128, 4, T], BF16)   # p = hl*64+d, head 2hp+hl
            kT = qkvp.tile([128, 4, T], BF16)
            v_sb = qkvp.tile([128, N_TT, HPC, D + 1], BF16)
            yT_sb = qkvp.tile([128, 4, T], BF16)

            # first-needed-first DMA order: xt chunk 0 interleaved with wqk,
            # then tables, then wv/wproj
            xt0 = xtp.tile([128, N_KO, TC], BF16, name="xt0", tag="xt")
            for ko in range(N_KO):
                nc.sync.dma_start(xt0[:, ko], xT_r[:, ko, 0:TC])
                nc.sync.dma_start(wqk_sb[:, ko], wqk_r[:, ko])
            nc.sync.dma_start(cos_sb[:], cosT_d[:])
            nc.sync.dma_start(sinN_sb[:], sinN_d[:])
            nc.sync.dma_start(mask_sb[:], mask_d[:])
            nc.sync.dma_start(id_sb[:], id_d[:])
            for ko in range(N_KO):
                nc.sync.dma_start(wv_sb[:, ko], wv_r[:, ko])
            nc.sync.dma_start(wp_sb[:], wproj_r[:])

            # ones columns of v (written once)
            nc.gpsimd.memset(v_sb[:, :, :, D], 1.0)

            for cch in range(N_TC):
                _phase_a_chunk(
                    nc, cch, xT_r, xt0 if cch == 0 else None,
                    xtp, ropep, ps_a, wqk_sb, wv_sb, cos_sb, sinN_sb,
                    qT, kT, v_sb,
                )
                _attention_ic(
                    nc, cch, qT, kT, v_sb, yT_sb, mask_sb, id_sb,
                    expp, finp, ps_s, ps_y, ps_o, [],
                )
                for f in _proj_units(nc, cch, yT_sb, wp_sb, ostp, ps_o,
                                     out_d, ps_y):
                    f()

    nc.compile()
    return nc


def _phase_a_units(nc, cch, xT_r, xtp, ropep, ps_a, wqk_sb, wv_sb,
                   cos_sb, sinN_sb, qT, kT, v_sb, xt0=None):
    """Return a list of closures, each emitting one qk column tile (GEMM +
    rope) or one v sub-tile for t-chunk cch."""
    ts_ = slice(cch * TC, (cch + 1) * TC)
    state = {}

    def get_xt():
        if "xt" not in state:
            if xt0 is not None:
                state["xt"] = xt0
            else:
                xt_sb = xtp.tile([128, N_KO, TC], BF16, name="xt", tag="xt")
                for ko in range(N_KO):
                    nc.sync.dma_start(xt_sb[:, ko], xT_r[:, ko, ts_])
                state["xt"] = xt_sb
        return state["xt"]

    def qk_unit(ct):
        def emit():
            xt_sb = get_xt()
            psum = ps_a.tile([128, TC], F32, name="ps_qk", tag="ps_a")
            for ko in range(N_KO):
                nc.tensor.matmul(
                    psum[:],
                    wqk_sb[:, ko, ct * 128:(ct + 1) * 128],
                    xt_sb[:, ko, :],
                    start=(ko == 0),
                    stop=(ko == N_KO - 1),
                )
            hp = ct % 4
            dest = (qT if ct < 4 else kT)[:, hp, ts_]
            # rope: raw copy out of PSUM, then 4x bf16 shuffle + mul + add
            raw = ropep.tile([128, TC], BF16, name="raw", tag="raw")
            with nc.allow_low_precision(reason="rope bf16"):
                nc.scalar.copy(raw[:], psum[:])
            rot = ropep.tile([128, TC], BF16, name="rot", tag="rot")
            for blk in range(4):
                src = (blk ^ 1) * 32
                nc.vector.tensor_copy(
                    rot[blk * 32: blk * 32 + 32, :], raw[src: src + 32, :]
                )
            t1 = ropep.tile([128, TC], BF16, name="t1", tag="t1")
            with nc.allow_low_precision(reason="rope bf16"):
                nc.vector.tensor_tensor(t1[:], raw[:], cos_sb[:, ts_], OP.mult)
                nc.vector.tensor_tensor(rot[:], rot[:], sinN_sb[:, ts_], OP.mult)
                nc.vector.tensor_tensor(dest, t1[:], rot[:], OP.add)
        return emit

    def v_unit(sub):
        def emit():
            xt_sb = get_xt()
            tt = cch * (TC // 128) + sub
            psv = ps_a.tile([128, HPC * D], F32, name="ps_v", tag="ps_a")
            for ko in range(N_KO):
                nc.tensor.matmul(
                    psv[:],
                    xt_sb[:, ko, sub * 128: sub * 128 + 128],
                    wv_sb[:, ko, :],
                    start=(ko == 0),
                    stop=(ko == N_KO - 1),
                )
            with nc.allow_low_precision(reason="v bf16"):
                nc.scalar.copy(
                    v_sb[:, tt, :, 0:D],
                    psv[:].rearrange("p (h d) -> p h d", d=D),
                )
        return emit

    # k and v tiles first: attention(cch) itself only needs q of chunk cch,
    # but attention(cch) woven around these units consumes k/v of chunk cch
    # only, so order within the chunk is free; keep qk first for rope flow.
    units = [qk_unit(ct) for ct in range(8)]
    units += [v_unit(sub) for sub in range(TC // 128)]
    return units


def _phase_a_chunk(nc, cch, xT_r, xt0, xtp, ropep, ps_a, wqk_sb, wv_sb,
                   cos_sb, sinN_sb, qT, kT, v_sb):
    for f in _phase_a_units(nc, cch, xT_r, xtp, ropep, ps_a, wqk_sb, wv_sb,
                            cos_sb, sinN_sb, qT, kT, v_sb, xt0=xt0):
        f()


def _attention_ic(nc, ic, qT, kT, v_sb, yT_sb, mask_sb, id_sb,
                  expp, finp, ps_s, ps_y, ps_o, filler):
    """Attention for query chunk ic (512 queries), all 4 head pairs.
    Pops one PE filler unit per (hp, hl) sub-loop to keep PE fed while
    ACT drains exp."""
    n_jt = (ic + 1) * 4
    is_ = slice(ic * IC, (ic + 1) * IC)
    # Bresenham-spread filler over the exp groups of this ic
    n_groups = 8 * (2 * ic + 2)
    n_fill = len(filler)
    gidx = [0]

    def pop_filler():
        want = ((gidx[0] + 1) * n_fill) // n_groups
        done = (gidx[0] * n_fill) // n_groups
        for _ in range(want - done):
            if filler:
                filler.pop(0)()
        gidx[0] += 1

    for hp in range(4):
        ytr = ps_o.tile([128, IC], BF16, name="ytr", tag="ps_o")
        for hl in range(2):
            pb = hl * 64
            h = 2 * hp + hl
            ypsum = ps_y.tile([128, 4, D + 1], F32, name="ypsum", tag="ypsum")
            # emit S+exp+AV jt-group-major so ACT drains while PE fills
            groups = []  # (psum_width, [(jt, off, w, masked)])
            for j0 in range(0, 4 * ic, 2):
                groups.append(
                    (1024, [(j0, 0, 512, False), (j0 + 1, 512, 512, False)])
                )
            groups.append(
                (1024, [(4 * ic, 0, 512, True), (4 * ic + 1, 512, 384, True)])
            )
            groups.append(
                (512, [(4 * ic + 2, 0, 256, True), (4 * ic + 3, 256, 128, True)])
            )
            def emit_s_exp(width, blocks):
                sp = ps_s.tile([128, 1024], F32, name="sp", tag="sp")
                for jt, off, w, masked in blocks:
                    lo = max(0, (jt - 4 * ic)) * 128
                    # start=True zeroes the whole 2KB PSUM bank: only the
                    # first block in each bank may set it
                    nc.tensor.matmul(
                        sp[:, off: off + w],
                        kT[pb: pb + 64, hp, jt * 128:(jt + 1) * 128],
                        qT[pb: pb + 64, hp, ic * IC + lo:(ic + 1) * IC],
                        start=(off % 512 == 0),
                        stop=not masked,
                        skip_group_check=True,
                    )
                    if masked:
                        nc.tensor.matmul(
                            sp[:, off: off + 128],
                            id_sb[:],
                            mask_sb[:],
                            start=False,
                            stop=True,
                            skip_group_check=True,
                        )
                ew = sum(b[2] for b in blocks)
                et = expp.tile([128, 1024], BF16, name="et", tag="et")
                with nc.allow_low_precision(reason="exp bf16"):
                    nc.scalar.activation(et[:, 0:ew], sp[:, 0:ew], AF.Exp)
                return et

            def emit_av(et, blocks):
                for jt, off, w, masked in blocks:
                    lo = max(0, (jt - 4 * ic)) * 128
                    for qs in range(4):
                        if qs * 128 < lo:
                            continue
                        col0 = off + qs * 128 - lo
                        nc.tensor.matmul(
                            ypsum[:, qs, :],
                            et[:, col0: col0 + 128],
                            v_sb[:, jt, h, :],
                            start=(jt == 0 and qs == 0),
                            stop=(jt == 4 * ic + qs),
                            skip_group_check=True,
                        )

            for width, blocks in groups:
                et = emit_s_exp(width, blocks)
                emit_av(et, blocks)
                pop_filler()
            # finalize: copy, recip, scale, transpose into ytr bank
            ysb = finp.tile([128, 4, D + 1], BF16, name="ysb", tag="ysb")
            nc.vector.tensor_copy(ysb[:], ypsum[:])
            rsb = finp.tile([128, 4], F32, name="rsb", tag="rsb")
            with nc.allow_low_precision(reason="softmax recip"):
                nc.vector.reciprocal(rsb[:], ysb[:, :, D])
            yn = finp.tile([128, 4, D], BF16, name="yn", tag="yn")
            with nc.allow_low_precision(reason="softmax scale bf16"):
                for qs in range(4):
                    nc.vector.tensor_scalar_mul(
                        yn[:, qs, :], ysb[:, qs, 0:D], rsb[:, qs: qs + 1]
                    )
            for qs in range(4):
                nc.tensor.matmul(
                    ytr[pb: pb + 64, qs * 128:(qs + 1) * 128],
                    yn[:, qs, :],
                    id_sb[:],
                    is_transpose=True,
                    start=(hl == 0 and qs == 0),
                    stop=(hl == 1 and qs == 3),
                    skip_group_check=True,
                )
        nc.vector.tensor_copy(yT_sb[:, hp, is_], ytr[:])


def _proj_units(nc, ic, yT_sb, wp_sb, ostp, ps_o, out_d, ps_y2=None):
    def unit(tt, cc, i):
        def emit():
            if ps_y2 is not None and i % 2 == 1:
                po = ps_y2.tile([128, 512], F32, name="po2", tag="ypsum")
            else:
                po = ps_o.tile([128, 512], F32, name="po", tag="ps_o")
            for hp in range(4):
                nc.tensor.matmul(
                    po[:],
                    yT_sb[:, hp, tt * 128:(tt + 1) * 128],
                    wp_sb[:, hp, cc * 512:(cc + 1) * 512],
                    start=(hp == 0),
                    stop=(hp == 3),
                )
            ost = ostp.tile([128, 512], BF16)
            with nc.allow_low_precision(reason="bf16 output"):
                nc.scalar.copy(ost[:], po[:])
            nc.sync.dma_start(
                out_d[tt * 128:(tt + 1) * 128, cc * 512:(cc + 1) * 512],
                ost[:],
            )
        return emit

    return [unit(tt, cc, i)
            for i, (tt, cc) in enumerate(
                (tt, cc)
                for tt in range(ic * 4, (ic + 1) * 4) for cc in range(2)
            )]


_NC = None
_STATE = None


def _get_nc():
    global _NC
    if _NC is None:
        _NC = _build()
    return _NC


def _tables_np():
    import ml_dtypes
    bf = ml_dtypes.bfloat16
    cosT, sinN = _rope_tables()
    return {
        "cosT": cosT.astype(bf),
        "sinN": sinN.astype(bf),
        "maskT": _mask_table().astype(bf),
        "ident": np.eye(128, dtype=np.float32).astype(bf),
    }


def _prep_w(W_attn, W_proj):
    """Per-head-group weight slices, bf16, quad-dedup'd (each core sends a
    quarter of its head-group's weights)."""
    import ml_dtypes
    bf = ml_dtypes.bfloat16
    scale = np.float32(1.0 / np.sqrt(D))
    wqk_hg, wv_hg, wp_hg = [], [], []
    for hg in range(2):
        cs = slice(hg * HPC * D, (hg + 1) * HPC * D)
        wq = W_attn[:, 0 * C:][:, cs] * scale
        wk = W_attn[:, 1 * C:][:, cs]
        wv = W_attn[:, 2 * C:][:, cs]
        wqk_hg.append(np.concatenate([wq, wk], axis=1).astype(bf))
        wv_hg.append(np.ascontiguousarray(wv).astype(bf))
        wp_hg.append(np.ascontiguousarray(W_proj[cs, :]).astype(bf))
    wqkc = np.empty((8 * 128, 1024), dtype=bf)
    wvc = np.empty((8 * 128, 512), dtype=bf)
    wpc = np.empty((8 * 64, C), dtype=bf)
    for c in range(8):
        q = c // 2
        wqkc[c * 128:(c + 1) * 128] = wqk_hg[c % 2][q * 128:(q + 1) * 128]
        wvc[c * 128:(c + 1) * 128] = wv_hg[c % 2][q * 128:(q + 1) * 128]
        wpc[c * 64:(c + 1) * 64] = wp_hg[c % 2][q * 64:(q + 1) * 64]
    return wqkc, wvc, wpc


def _prep_compact(x, W_attn, W_proj):
    import ml_dtypes
    xs = np.ascontiguousarray(x.reshape(8 * (T // 2), C)).astype(
        ml_dtypes.bfloat16
    )
    wqkc, wvc, wpc = _prep_w(W_attn, W_proj)
    return xs, wqkc, wvc, wpc


def _get_state():
    global _STATE
    if _STATE is not None:
        return _STATE

    import jax
    import jax.numpy as jnp
    from jax.experimental.shard_map import shard_map
    from jax.sharding import Mesh, NamedSharding, PartitionSpec

    from concourse import bass2jax

    nc = _get_nc()
    bass2jax.install_neuronx_cc_hook()
    partition_name = nc.partition_id_tensor.name if nc.partition_id_tensor else None
    in_names, out_names, out_avals = [], [], []
    for alloc in nc.m.functions[0].allocations:
        if not isinstance(alloc, mybir.MemoryLocationSet):
            continue
        name = alloc.memorylocations[0].name
        if alloc.kind == "ExternalInput":
            if name != partition_name:
                in_names.append(name)
        elif alloc.kind == "ExternalOutput":
            out_names.append(name)
            out_avals.append(
                jax.core.ShapedArray(tuple(alloc.tensor_shape), mybir.dt.np(alloc.dtype))
            )
    n_params, n_outs = len(in_names), len(out_avals)
    all_names = list(in_names) + out_names
    if partition_name:
        all_names.append(partition_name)

    def _body(*args):
        operands = list(args)
        if partition_name:
            operands.append(bass2jax.partition_id_tensor())
        outs = bass2jax._bass_exec_p.bind(
            *operands,
            out_avals=tuple(out_avals),
            in_names=tuple(all_names),
            out_names=tuple(out_names),
            lowering_input_output_aliases=(),
            sim_require_finite=True,
            sim_require_nnan=True,
            nc=nc,
        )
        return tuple(outs)

    devices = jax.devices()[:8]
    mesh = Mesh(np.asarray(devices), ("core",))
    shd = NamedSharding(mesh, PartitionSpec("core"))
    donate = tuple(range(n_params, n_params + n_outs))
    sharded = jax.jit(
        shard_map(
            _body,
            mesh=mesh,
            in_specs=(PartitionSpec("core"),) * (n_params + n_outs),
            out_specs=(PartitionSpec("core"),) * n_outs,
            check_rep=False,
        ),
        donate_argnums=donate,
        keep_unused=True,
    )
    zeros_fn = jax.jit(
        lambda: tuple(
            jnp.zeros((8 * av.shape[0],) + av.shape[1:], av.dtype) for av in out_avals
        ),
        out_shardings=(shd,) * n_outs,
    )

    tabs = _tables_np()
    statics = {
        k: jax.device_put(np.tile(v, (8, 1)), shd) for k, v in tabs.items()
    }
    jax.block_until_ready(list(statics.values()))

    PAIRS = [[0, 1], [2, 3], [4, 5], [6, 7]]
    QUADS = [[0, 2, 4, 6], [1, 3, 5, 7]]

    def _pre(xs, wqk, wv, wp):
        xg = jax.lax.all_gather(xs, "core", axis_index_groups=PAIRS, axis=0, tiled=True)
        wqkg = jax.lax.all_gather(wqk, "core", axis_index_groups=QUADS, axis=0, tiled=True)
        wvg = jax.lax.all_gather(wv, "core", axis_index_groups=QUADS, axis=0, tiled=True)
        wpg = jax.lax.all_gather(wp, "core", axis_index_groups=QUADS, axis=0, tiled=True)
        zeros = tuple(jnp.zeros(av.shape, av.dtype) for av in out_avals)
        return (xg.T, wqkg, wvg, wpg) + zeros

    pre_fn = jax.jit(
        shard_map(
            _pre,
            mesh=mesh,
            in_specs=(PartitionSpec("core"),) * 4,
            out_specs=(PartitionSpec("core"),) * (4 + n_outs),
        )
    )

    def _post(o):
        other = jax.lax.ppermute(
            o, "core", [(0, 1), (1, 0), (2, 3), (3, 2), (4, 5), (5, 4), (6, 7), (7, 6)]
        )
        s = o.astype(jnp.float32) + other.astype(jnp.float32)
        idx = jax.lax.axis_index("core")
        return jax.lax.dynamic_slice(s, ((idx % 2) * (T // 2), 0), (T // 2, C))

    post_fn = jax.jit(
        shard_map(
            _post,
            mesh=mesh,
            in_specs=(PartitionSpec("core"),),
            out_specs=PartitionSpec("core"),
        )
    )

    _STATE = dict(
        jax=jax,
        nc=nc,
        in_names=in_names,
        out_names=out_names,
        n_outs=n_outs,
        sharded=sharded,
        zeros_fn=zeros_fn,
        shd=shd,
        statics=statics,
        pre_fn=pre_fn,
        post_fn=post_fn,
    )
    return _STATE


def _run_gathered(st, x, W_attn, W_proj):
    jax = st["jax"]
    import ml_dtypes
    xs = np.ascontiguousarray(x.reshape(8 * (T // 2), C)).astype(
        ml_dtypes.bfloat16
    )
    d_xs = jax.device_put(xs, st["shd"])
    wqkc, wvc, wpc = _prep_w(W_attn, W_proj)
    d_wqk = jax.device_put(wqkc, st["shd"])
    d_wv = jax.device_put(wvc, st["shd"])
    d_wp = jax.device_put(wpc, st["shd"])
    pre = st["pre_fn"](d_xs, d_wqk, d_wv, d_wp)
    dyn = {"xT": pre[0], "wqk": pre[1], "wv": pre[2], "wproj": pre[3]}
    args = [dyn[nm] if nm in dyn else st["statics"][nm] for nm in st["in_names"]]
    outs = st["sharded"](*args, *pre[4:])
    po = st["post_fn"](outs[0])
    r = np.asarray(po).reshape(B, T, C)
    return np.ascontiguousarray(r)


def _run_rbks(x, W_attn, W_proj):
    """Fallback: the stock run_bass_kernel_spmd entry point."""
    import ml_dtypes
    bf = ml_dtypes.bfloat16
    nc = _get_nc()
    tabs = _tables_np()
    scale = np.float32(1.0 / np.sqrt(D))
    in_maps = []
    for core in range(8):
        b, hg = core // 2, core % 2
        cs = slice(hg * HPC * D, (hg + 1) * HPC * D)
        wq = W_attn[:, 0 * C:][:, cs] * scale
        wk = W_attn[:, 1 * C:][:, cs]
        wv = W_attn[:, 2 * C:][:, cs]
        m = {
            "xT": np.ascontiguousarray(x[b].T).astype(bf),
            "wqk": np.concatenate([wq, wk], axis=1).astype(bf),
            "wv": np.ascontiguousarray(wv).astype(bf),
            "wproj": np.ascontiguousarray(W_proj[cs, :]).astype(bf),
        }
        m.update(tabs)
        in_maps.append(m)
    res = run_bass_kernel_spmd(nc, in_maps, core_ids=list(range(8)))
    out = np.empty((B, T, C), dtype=np.float32)
    for b in range(B):
        out[b] = res.results[2 * b]["out"].astype(np.float32) + res.results[
            2 * b + 1
        ]["out"].astype(np.float32)
    return out


def kernel(x, W_attn, W_proj):
    x = np.asarray(x, dtype=np.float32)
    W_attn = np.asarray(W_attn, dtype=np.float32)
    W_proj = np.asarray(W_proj, dtype=np.float32)

    try:
        st = _get_state()
        return _run_gathered(st, x, W_attn, W_proj)
    except Exception:
        return _run_rbks(x, W_attn, W_proj)


if __name__ == "__main__":
    nc = _get_nc()
    from concourse.timeline_sim import TimelineSim
    sim_ns = TimelineSim(nc, trace=False).simulate()
    print(f"timeline-sim: {sim_ns/1e3:.1f} us")


# revision 5
# speedup vs baseline: 1.0509x; 1.0300x over previous
"""Causal self-attention with RoPE on 8 Trainium2 NeuronCores — v4.

Full inputs: x [4, 2048, 1024], W_attn [1024, 3072], W_proj [1024, 1024] (f32).
Sharding: core = b*2 + hg  (4 batches x 2 head-groups of 8 heads).
Host sums the two head-group partials per batch.

v4 design (all-bf16 data path):
- qk GEMM bf16 (stationary W tiles, moving xT), v GEMM bf16.
- RoPE: one PSUM->SBUF copy, then 4x-mode bf16 shuffle/mul/add on DVE.
- S = K^T Q per (head, 128-key tile) in bf16; causal masking by accumulating
  an additive -30000 mask block into PSUM via a second matmul (identity
  stationary), so exp() of masked entries is exactly 0.
- exp: batched ACT instructions over a [128, 1024] PSUM pair-arena with the
  diagonal blocks packed at exact widths (no wasted exp columns).
- AV transposed: stationary = expT tile [128k, 128q], moving = v [128k, 65]
  (65th column of v is ones -> denominator lands in output col 64).
- y finalize: copy/recip/scale (DVE) then PE transpose back to [d, t] layout.
- Output projection bf16, output DMA'd as bf16 and upcast on host.
"""

import sys

sys.path.insert(0, "/opt/trn_rl_repo")

import numpy as np

import concourse.bass as bass  # noqa: F401
import concourse.mybir as mybir
import concourse.tile as tile
from concourse import bacc
from concourse.bass_utils import run_bass_kernel_spmd

F32 = mybir.dt.float32
BF16 = mybir.dt.bfloat16
FP8 = mybir.dt.float8e4
AF = mybir.ActivationFunctionType
OP = mybir.AluOpType

B, T, C = 4, 2048, 1024
H, D = 16, 64
HPC = 8                 # heads per core
N_KO = C // 128         # 8 contraction chunks
TC = 512                # t-chunk width in phase A
N_TC = T // TC          # 4
N_TT = T // 128         # 16 key tiles
N_IC = 4                # i-chunks of 512 queries
IC = 512
NEG = -30000.0

# diag pack: widths of the 4 diagonal jt blocks and their column offsets in
# the two diag psum tiles (A: [512|384] -> 896 cols, B: [256|128] -> 384)
DIAG_W = [512, 384, 256, 128]
DIAG_TILE = [0, 0, 1, 1]        # which diag tile
DIAG_OFF = [0, 512, 0, 256]     # col offset within its tile
DIAG_TW = [896, 384]            # widths of diag tiles A and B


def _rope_tables():
    """cosT/sinN [128, T]: row p holds tables for d = p % 64; sinN has the
    rotate-half sign folded in (rows with d%64 < 32 negative)."""
    inv_freq = (
        np.float32(1.0)
        / np.float32(10000.0) ** (np.arange(0, D, 2, dtype=np.float32) / np.float32(D))
    ).astype(np.float32)
    t = np.arange(T, dtype=np.float32)
    freqs = (t[:, None] * inv_freq[None, :]).astype(np.float32)  # [T, 32]
    emb = np.concatenate([freqs, freqs], axis=1)  # [T, 64]
    cos = np.cos(emb).astype(np.float32)
    sin = np.sin(emb).astype(np.float32)
    sinN = np.concatenate([-sin[:, :32], sin[:, 32:]], axis=1)
    cosT = np.tile(cos.T, (2, 1))   # [128, T]
    sinNT = np.tile(sinN.T, (2, 1))
    return np.ascontiguousarray(cosT), np.ascontiguousarray(sinNT)


def _mask_table():
    """maskT [128, 128]: 0.0 if k <= q else NEG  (S^T layout: [keys, queries])."""
    k = np.arange(128)[:, None]
    q = np.arange(128)[None, :]
    return np.where(k <= q, 0.0, NEG).astype(np.float32)


def _build():
    nc = bacc.Bacc(None, target_bir_lowering=False, debug=False)

    xT = nc.dram_tensor("xT", [C, T], BF16, kind="ExternalInput")
    x8 = nc.dram_tensor("x8", [C, T], FP8, kind="ExternalInput")
    xlo8 = nc.dram_tensor("xlo8", [C, T], FP8, kind="ExternalInput")
    w8 = nc.dram_tensor("w8", [C, 1024], FP8, kind="ExternalInput")
    w2_8 = nc.dram_tensor("w2_8", [C, 1024], FP8, kind="ExternalInput")
    wlo8 = nc.dram_tensor("wlo8", [C, 1024], FP8, kind="ExternalInput")
    wv = nc.dram_tensor("wv", [C, 512], BF16, kind="ExternalInput")
    wproj = nc.dram_tensor("wproj", [HPC * D, C], BF16, kind="ExternalInput")
    cosT_d = nc.dram_tensor("cosT", [128, T], BF16, kind="ExternalInput")
    sinN_d = nc.dram_tensor("sinN", [128, T], BF16, kind="ExternalInput")
    mask_d = nc.dram_tensor("maskT", [128, 128], BF16, kind="ExternalInput")
    id_d = nc.dram_tensor("ident", [128, 128], BF16, kind="ExternalInput")
    out_d = nc.dram_tensor("out", [T, C], BF16, kind="ExternalOutput")

    xT_r = xT.rearrange("(ko p) t -> p ko t", p=128)
    x8_r = x8.rearrange("(ko2 two p) t -> p ko2 two t", p=128, two=2)
    xlo8_r = xlo8.rearrange("(ko2 two p) t -> p ko2 two t", p=128, two=2)
    w8_r = w8.rearrange("(ko2 two p) c -> p ko2 two c", p=128, two=2)
    w2_r = w2_8.rearrange("(ko2 two p) c -> p ko2 two c", p=128, two=2)
    wlo_r = wlo8.rearrange("(ko2 two p) c -> p ko2 two c", p=128, two=2)
    wv_r = wv.rearrange("(ko p) c -> p ko c", p=128)
    wproj_r = wproj.rearrange("(hp p) c -> p hp c", p=128)

    with tile.TileContext(nc) as tc:
        with (
            tc.tile_pool(name="res", bufs=1) as res,
            tc.tile_pool(name="qkv", bufs=1) as qkvp,
            tc.tile_pool(name="xt", bufs=2) as xtp,
            tc.tile_pool(name="rope", bufs=4) as ropep,
            tc.tile_pool(name="exp", bufs=4) as expp,
            tc.tile_pool(name="fin", bufs=3) as finp,
            tc.tile_pool(name="ost", bufs=4) as ostp,
            tc.tile_pool(name="ps_a", bufs=2, space="PSUM") as ps_a,
            tc.tile_pool(name="ps_s", bufs=2, space="PSUM") as ps_s,
            tc.tile_pool(name="ps_y", bufs=1, space="PSUM") as ps_y,
            tc.tile_pool(name="ps_o", bufs=1, space="PSUM") as ps_o,
        ):
            # ---- resident tables / weights ----
            cos_sb = res.tile([128, T], BF16)
            sinN_sb = res.tile([128, T], BF16)
            mask_sb = res.tile([128, 128], BF16)
            id_sb = res.tile([128, 128], BF16)
            w8_sb = res.tile([128, 4, 2, 1024], FP8)
            w2_sb = res.tile([128, 4, 2, 1024], FP8)
            wlo_sb = res.tile([128, 4, 2, 1024], FP8)
            wv_sb = res.tile([128, N_KO, 512], BF16)
            wp_sb = res.tile([128, 4, C], BF16)

            # ---- phase A outputs ----
            qT = qkvp.tile([128, 4, T], BF16)   # p = hl*64+d, head 2hp+hl
            kT = qkvp.tile([128, 4, T], BF16)
            v_sb = qkvp.tile([128, N_TT, HPC, D + 1], BF16)
            yT_sb = qkvp.tile([128, 4, T], BF16)

            # first-needed-first DMA order: xt chunk 0 interleaved with wqk,
            # then tables, then wv/wproj
            xt0 = xtp.tile([128, N_KO, TC], BF16, name="xt0", tag="xt")
            x80 = xtp.tile([128, 4, 2, TC], FP8, name="x80", tag="x8")
            xlo0 = xtp.tile([128, 4, 2, TC], FP8, name="xlo0", tag="xlo")
            for ko2 in range(4):
                nc.sync.dma_start(x80[:, ko2], x8_r[:, ko2, :, 0:TC])
                nc.sync.dma_start(xlo0[:, ko2], xlo8_r[:, ko2, :, 0:TC])
                nc.sync.dma_start(w8_sb[:, ko2], w8_r[:, ko2])
                nc.sync.dma_start(w2_sb[:, ko2], w2_r[:, ko2])
                nc.sync.dma_start(wlo_sb[:, ko2], wlo_r[:, ko2])
            for ko in range(N_KO):
                nc.sync.dma_start(xt0[:, ko], xT_r[:, ko, 0:TC])
            nc.sync.dma_start(cos_sb[:], cosT_d[:])
            nc.sync.dma_start(sinN_sb[:], sinN_d[:])
            nc.sync.dma_start(mask_sb[:], mask_d[:])
            nc.sync.dma_start(id_sb[:], id_d[:])
            for ko in range(N_KO):
                nc.sync.dma_start(wv_sb[:, ko], wv_r[:, ko])
            nc.sync.dma_start(wp_sb[:], wproj_r[:])

            # ones columns of v (written once)
            nc.gpsimd.memset(v_sb[:, :, :, D], 1.0)

            wsets = (w8_sb, w2_sb, wlo_sb)
            xrs = (xT_r, x8_r, xlo8_r)
            for cch in range(N_TC):
                _phase_a_chunk(
                    nc, cch, xrs, (xt0, x80, xlo0) if cch == 0 else None,
                    xtp, ropep, ps_a, wsets, wv_sb, cos_sb, sinN_sb,
                    qT, kT, v_sb,
                )
                _attention_ic(
                    nc, cch, qT, kT, v_sb, yT_sb, mask_sb, id_sb,
                    expp, finp, ps_s, ps_y, ps_o, [],
                )
                for f in _proj_units(nc, cch, yT_sb, wp_sb, ostp, ps_o,
                                     out_d, ps_y):
                    f()

    nc.compile()
    return nc


def _phase_a_units(nc, cch, xrs, xtp, ropep, ps_a, wsets, wv_sb,
                   cos_sb, sinN_sb, qT, kT, v_sb, xt0=None):
    """Return a list of closures, each emitting one qk column tile (GEMM +
    rope) or one v sub-tile for t-chunk cch."""
    xT_r, x8_r, xlo8_r = xrs
    w8_sb, w2_sb, wlo_sb = wsets
    ts_ = slice(cch * TC, (cch + 1) * TC)
    state = {}

    def get_xt():
        if "xt" not in state:
            if xt0 is not None:
                state["xt"] = xt0
            else:
                xt_sb = xtp.tile([128, N_KO, TC], BF16, name="xt", tag="xt")
                x8t = xtp.tile([128, 4, 2, TC], FP8, name="x8t", tag="x8")
                xlot = xtp.tile([128, 4, 2, TC], FP8, name="xlot", tag="xlo")
                for ko2 in range(4):
                    nc.sync.dma_start(x8t[:, ko2], x8_r[:, ko2, :, ts_])
                    nc.sync.dma_start(xlot[:, ko2], xlo8_r[:, ko2, :, ts_])
                for ko in range(N_KO):
                    nc.sync.dma_start(xt_sb[:, ko], xT_r[:, ko, ts_])
                state["xt"] = (xt_sb, x8t, xlot)
        return state["xt"]

    def qk_unit(ct):
        def emit():
            _, x8t, xlot = get_xt()
            cs = slice(ct * 128, (ct + 1) * 128)
            psum = ps_a.tile([128, TC], F32, name="ps_qk", tag="ps_a")
            # 3-term error-compensated fp8 DoubleRow GEMM:
            #   x8@W8 + xlo8@(W/16)8 + x8@(W - W8)8  ~=  x @ 32W
            # moving free limit 512 -> two 256-wide t halves; both halves
            # share one PSUM bank so only the very first matmul starts it
            first = True
            for th in range(2):
                h0 = th * 256
                terms = ((x8t, w8_sb), (xlot, w2_sb), (x8t, wlo_sb))
                for ti, (xm, wm) in enumerate(terms):
                    for ko2 in range(4):
                        nc.tensor.matmul(
                            psum[:, h0: h0 + 256],
                            wm[:, ko2, :, cs],
                            xm[:, ko2, :, h0: h0 + 256],
                            start=first,
                            stop=(ti == 2 and ko2 == 3),
                            perf_mode=mybir.MatmulPerfMode.DoubleRow,
                            skip_group_check=True,
                        )
                        first = False
            hp = ct % 4
            dest = (qT if ct < 4 else kT)[:, hp, ts_]
            # rope: raw copy out of PSUM, then 4x bf16 shuffle + mul + add
            raw = ropep.tile([128, TC], BF16, name="raw", tag="raw")
            with nc.allow_low_precision(reason="rope bf16"):
                nc.scalar.copy(raw[:], psum[:])
            rot = ropep.tile([128, TC], BF16, name="rot", tag="rot")
            for blk in range(4):
                src = (blk ^ 1) * 32
                nc.vector.tensor_copy(
                    rot[blk * 32: blk * 32 + 32, :], raw[src: src + 32, :]
                )
            t1 = ropep.tile([128, TC], BF16, name="t1", tag="t1")
            with nc.allow_low_precision(reason="rope bf16"):
                nc.vector.tensor_tensor(t1[:], raw[:], cos_sb[:, ts_], OP.mult)
                nc.vector.tensor_tensor(rot[:], rot[:], sinN_sb[:, ts_], OP.mult)
                nc.vector.tensor_tensor(dest, t1[:], rot[:], OP.add)
        return emit

    def v_unit(sub):
        def emit():
            xt_sb = get_xt()[0]
            tt = cch * (TC // 128) + sub
            psv = ps_a.tile([128, HPC * D], F32, name="ps_v", tag="ps_a")
            for ko in range(N_KO):
                nc.tensor.matmul(
                    psv[:],
                    xt_sb[:, ko, sub * 128: sub * 128 + 128],
                    wv_sb[:, ko, :],
                    start=(ko == 0),
                    stop=(ko == N_KO - 1),
                )
            with nc.allow_low_precision(reason="v bf16"):
                nc.scalar.copy(
                    v_sb[:, tt, :, 0:D],
                    psv[:].rearrange("p (h d) -> p h d", d=D),
                )
        return emit

    # k and v tiles first: attention(cch) itself only needs q of chunk cch,
    # but attention(cch) woven around these units consumes k/v of chunk cch
    # only, so order within the chunk is free; keep qk first for rope flow.
    units = [qk_unit(ct) for ct in range(8)]
    units += [v_unit(sub) for sub in range(TC // 128)]
    return units


def _phase_a_chunk(nc, cch, xrs, xt0, xtp, ropep, ps_a, wsets, wv_sb,
                   cos_sb, sinN_sb, qT, kT, v_sb):
    for f in _phase_a_units(nc, cch, xrs, xtp, ropep, ps_a, wsets, wv_sb,
                            cos_sb, sinN_sb, qT, kT, v_sb, xt0=xt0):
        f()


def _attention_ic(nc, ic, qT, kT, v_sb, yT_sb, mask_sb, id_sb,
                  expp, finp, ps_s, ps_y, ps_o, filler):
    """Attention for query chunk ic (512 queries), all 4 head pairs.
    Pops one PE filler unit per (hp, hl) sub-loop to keep PE fed while
    ACT drains exp."""
    n_jt = (ic + 1) * 4
    is_ = slice(ic * IC, (ic + 1) * IC)
    # Bresenham-spread filler over the exp groups of this ic
    n_groups = 8 * (2 * ic + 2)
    n_fill = len(filler)
    gidx = [0]

    def pop_filler():
        want = ((gidx[0] + 1) * n_fill) // n_groups
        done = (gidx[0] * n_fill) // n_groups
        for _ in range(want - done):
            if filler:
                filler.pop(0)()
        gidx[0] += 1

    for hp in range(4):
        ytr = ps_o.tile([128, IC], BF16, name="ytr", tag="ps_o")
        for hl in range(2):
            pb = hl * 64
            h = 2 * hp + hl
            ypsum = ps_y.tile([128, 4, D + 1], F32, name="ypsum", tag="ypsum")
            # emit S+exp+AV jt-group-major so ACT drains while PE fills
            groups = []  # (psum_width, [(jt, off, w, masked)])
            for j0 in range(0, 4 * ic, 2):
                groups.append(
                    (1024, [(j0, 0, 512, False), (j0 + 1, 512, 512, False)])
                )
            groups.append(
                (1024, [(4 * ic, 0, 512, True), (4 * ic + 1, 512, 384, True)])
            )
            groups.append(
                (512, [(4 * ic + 2, 0, 256, True), (4 * ic + 3, 256, 128, True)])
            )
            def emit_s_exp(width, blocks):
                sp = ps_s.tile([128, 1024], F32, name="sp", tag="sp")
                for jt, off, w, masked in blocks:
                    lo = max(0, (jt - 4 * ic)) * 128
                    # start=True zeroes the whole 2KB PSUM bank: only the
                    # first block in each bank may set it
                    nc.tensor.matmul(
                        sp[:, off: off + w],
                        kT[pb: pb + 64, hp, jt * 128:(jt + 1) * 128],
                        qT[pb: pb + 64, hp, ic * IC + lo:(ic + 1) * IC],
                        start=(off % 512 == 0),
                        stop=not masked,
                        skip_group_check=True,
                    )
                    if masked:
                        nc.tensor.matmul(
                            sp[:, off: off + 128],
                            id_sb[:],
                            mask_sb[:],
                            start=False,
                            stop=True,
                            skip_group_check=True,
                        )
                ew = sum(b[2] for b in blocks)
                et = expp.tile([128, 1024], BF16, name="et", tag="et")
                with nc.allow_low_precision(reason="exp bf16"):
                    nc.scalar.activation(et[:, 0:ew], sp[:, 0:ew], AF.Exp)
                return et

            def emit_av(et, blocks):
                for jt, off, w, masked in blocks:
                    lo = max(0, (jt - 4 * ic)) * 128
                    for qs in range(4):
                        if qs * 128 < lo:
                            continue
                        col0 = off + qs * 128 - lo
                        nc.tensor.matmul(
                            ypsum[:, qs, :],
                            et[:, col0: col0 + 128],
                            v_sb[:, jt, h, :],
                            start=(jt == 0 and qs == 0),
                            stop=(jt == 4 * ic + qs),
                            skip_group_check=True,
                        )

            for width, blocks in groups:
                et = emit_s_exp(width, blocks)
                emit_av(et, blocks)
                pop_filler()
            # finalize: copy, recip, scale, transpose into ytr bank
            ysb = finp.tile([128, 4, D + 1], BF16, name="ysb", tag="ysb")
            nc.vector.tensor_copy(ysb[:], ypsum[:])
            rsb = finp.tile([128, 4], F32, name="rsb", tag="rsb")
            with nc.allow_low_precision(reason="softmax recip"):
                nc.vector.reciprocal(rsb[:], ysb[:, :, D])
            yn = finp.tile([128, 4, D], BF16, name="yn", tag="yn")
            with nc.allow_low_precision(reason="softmax scale bf16"):
                for qs in range(4):
                    nc.vector.tensor_scalar_mul(
                        yn[:, qs, :], ysb[:, qs, 0:D], rsb[:, qs: qs + 1]
                    )
            for qs in range(4):
                nc.tensor.matmul(
                    ytr[pb: pb + 64, qs * 128:(qs + 1) * 128],
                    yn[:, qs, :],
                    id_sb[:],
                    is_transpose=True,
                    start=(hl == 0 and qs == 0),
                    stop=(hl == 1 and qs == 3),
                    skip_group_check=True,
                )
        nc.vector.tensor_copy(yT_sb[:, hp, is_], ytr[:])


def _proj_units(nc, ic, yT_sb, wp_sb, ostp, ps_o, out_d, ps_y2=None):
    def unit(tt, cc, i):
        def emit():
            if ps_y2 is not None and i % 2 == 1:
                po = ps_y2.tile([128, 512], F32, name="po2", tag="ypsum")
            else:
                po = ps_o.tile([128, 512], F32, name="po", tag="ps_o")
            for hp in range(4):
                nc.tensor.matmul(
                    po[:],
                    yT_sb[:, hp, tt * 128:(tt + 1) * 128],
                    wp_sb[:, hp, cc * 512:(cc + 1) * 512],
                    start=(hp == 0),
                    stop=(hp == 3),
                )
            ost = ostp.tile([128, 512], BF16)
            with nc.allow_low_precision(reason="bf16 output"):
                nc.scalar.copy(ost[:], po[:])
            nc.sync.dma_start(
                out_d[tt * 128:(tt + 1) * 128, cc * 512:(cc + 1) * 512],
                ost[:],
            )
        return emit

    return [unit(tt, cc, i)
            for i, (tt, cc) in enumerate(
                (tt, cc)
                for tt in range(ic * 4, (ic + 1) * 4) for cc in range(2)
            )]


_NC = None
_STATE = None


def _get_nc():
    global _NC
    if _NC is None:
        _NC = _build()
    return _NC


def _tables_np():
    import ml_dtypes
    bf = ml_dtypes.bfloat16
    cosT, sinN = _rope_tables()
    # qk psums carry a 32x fp8 scale; fold 1/32 into the rope tables
    return {
        "cosT": (cosT / 32.0).astype(bf),
        "sinN": (sinN / 32.0).astype(bf),
        "maskT": _mask_table().astype(bf),
        "ident": np.eye(128, dtype=np.float32).astype(bf),
    }


def _np_fp8():
    return mybir.dt.np(FP8)


def _to_fp8(a):
    return np.clip(a, -240.0, 240.0).astype(_np_fp8())


def _prep_w(W_attn, W_proj):
    """Per-head-group weight slices, quad-dedup'd. qk weights go out as
    three fp8 tensors (32x scaled main + two residual-compensation sets)."""
    import ml_dtypes
    bf = ml_dtypes.bfloat16
    scale = np.float32(1.0 / np.sqrt(D))
    w8_hg, w2_hg, wlo_hg, wv_hg, wp_hg = [], [], [], [], []
    for hg in range(2):
        cs = slice(hg * HPC * D, (hg + 1) * HPC * D)
        wq = W_attn[:, 0 * C:][:, cs] * scale
        wk = W_attn[:, 1 * C:][:, cs]
        wv = W_attn[:, 2 * C:][:, cs]
        w32 = np.concatenate([wq, wk], axis=1) * np.float32(32.0)
        w8 = _to_fp8(w32)
        w8_hg.append(w8)
        w2_hg.append(_to_fp8(w32 / 16.0))
        wlo_hg.append(_to_fp8(w32 - w8.astype(np.float32)))
        wv_hg.append(np.ascontiguousarray(wv).astype(bf))
        wp_hg.append(np.ascontiguousarray(W_proj[cs, :]).astype(bf))
    f8 = _np_fp8()
    w8c = np.empty((8 * 128, 1024), dtype=f8)
    w2c = np.empty((8 * 128, 1024), dtype=f8)
    wloc = np.empty((8 * 128, 1024), dtype=f8)
    wvc = np.empty((8 * 128, 512), dtype=bf)
    wpc = np.empty((8 * 64, C), dtype=bf)
    for c in range(8):
        q = c // 2
        sl = slice(c * 128, (c + 1) * 128)
        qs = slice(q * 128, (q + 1) * 128)
        w8c[sl] = w8_hg[c % 2][qs]
        w2c[sl] = w2_hg[c % 2][qs]
        wloc[sl] = wlo_hg[c % 2][qs]
        wvc[sl] = wv_hg[c % 2][qs]
        wpc[c * 64:(c + 1) * 64] = wp_hg[c % 2][q * 64:(q + 1) * 64]
    return w8c, w2c, wloc, wvc, wpc


def _prep_x8(x):
    """fp8 x and its scaled residual, pre-transposed per batch; core c
    sends the (c%2) column-half of batch c//2's xT."""
    f8 = _np_fp8()
    x8s = np.empty((8 * C, T // 2), dtype=f8)
    xlos = np.empty((8 * C, T // 2), dtype=f8)
    for b in range(B):
        x8b = _to_fp8(x[b])
        xlob = _to_fp8(16.0 * (x[b] - x8b.astype(np.float32)))
        x8T = np.ascontiguousarray(x8b.T)
        xloT = np.ascontiguousarray(xlob.T)
        for half in range(2):
            c = 2 * b + half
            hs = slice(half * (T // 2), (half + 1) * (T // 2))
            x8s[c * C:(c + 1) * C] = x8T[:, hs]
            xlos[c * C:(c + 1) * C] = xloT[:, hs]
    return x8s, xlos


def _prep_compact(x, W_attn, W_proj):
    import ml_dtypes
    xs = np.ascontiguousarray(x.reshape(8 * (T // 2), C)).astype(
        ml_dtypes.bfloat16
    )
    x8s, xlos = _prep_x8(x)
    return (xs, x8s, xlos) + _prep_w(W_attn, W_proj)


def _get_state():
    global _STATE
    if _STATE is not None:
        return _STATE

    import jax
    import jax.numpy as jnp
    from jax.experimental.shard_map import shard_map
    from jax.sharding import Mesh, NamedSharding, PartitionSpec

    from concourse import bass2jax

    nc = _get_nc()
    bass2jax.install_neuronx_cc_hook()
    partition_name = nc.partition_id_tensor.name if nc.partition_id_tensor else None
    in_names, out_names, out_avals = [], [], []
    for alloc in nc.m.functions[0].allocations:
        if not isinstance(alloc, mybir.MemoryLocationSet):
            continue
        name = alloc.memorylocations[0].name
        if alloc.kind == "ExternalInput":
            if name != partition_name:
                in_names.append(name)
        elif alloc.kind == "ExternalOutput":
            out_names.append(name)
            out_avals.append(
                jax.core.ShapedArray(tuple(alloc.tensor_shape), mybir.dt.np(alloc.dtype))
            )
    n_params, n_outs = len(in_names), len(out_avals)
    all_names = list(in_names) + out_names
    if partition_name:
        all_names.append(partition_name)

    def _body(*args):
        operands = list(args)
        if partition_name:
            operands.append(bass2jax.partition_id_tensor())
        outs = bass2jax._bass_exec_p.bind(
            *operands,
            out_avals=tuple(out_avals),
            in_names=tuple(all_names),
            out_names=tuple(out_names),
            lowering_input_output_aliases=(),
            sim_require_finite=True,
            sim_require_nnan=True,
            nc=nc,
        )
        return tuple(outs)

    devices = jax.devices()[:8]
    mesh = Mesh(np.asarray(devices), ("core",))
    shd = NamedSharding(mesh, PartitionSpec("core"))
    donate = tuple(range(n_params, n_params + n_outs))
    sharded = jax.jit(
        shard_map(
            _body,
            mesh=mesh,
            in_specs=(PartitionSpec("core"),) * (n_params + n_outs),
            out_specs=(PartitionSpec("core"),) * n_outs,
            check_rep=False,
        ),
        donate_argnums=donate,
        keep_unused=True,
    )
    zeros_fn = jax.jit(
        lambda: tuple(
            jnp.zeros((8 * av.shape[0],) + av.shape[1:], av.dtype) for av in out_avals
        ),
        out_shardings=(shd,) * n_outs,
    )

    tabs = _tables_np()
    statics = {
        k: jax.device_put(np.tile(v, (8, 1)), shd) for k, v in tabs.items()
    }
    jax.block_until_ready(list(statics.values()))

    PAIRS = [[0, 1], [2, 3], [4, 5], [6, 7]]
    QUADS = [[0, 2, 4, 6], [1, 3, 5, 7]]

    def _pre(xs, x8s, xlos, w8, w2, wlo, wv, wp):
        gq = lambda a: jax.lax.all_gather(
            a, "core", axis_index_groups=QUADS, axis=0, tiled=True)
        xg = jax.lax.all_gather(xs, "core", axis_index_groups=PAIRS, axis=0, tiled=True)
        x8g = jax.lax.all_gather(x8s, "core", axis_index_groups=PAIRS, axis=1, tiled=True)
        xlog = jax.lax.all_gather(xlos, "core", axis_index_groups=PAIRS, axis=1, tiled=True)
        zeros = tuple(jnp.zeros(av.shape, av.dtype) for av in out_avals)
        return (xg.T, x8g, xlog, gq(w8), gq(w2), gq(wlo), gq(wv), gq(wp)) + zeros

    pre_fn = jax.jit(
        shard_map(
            _pre,
            mesh=mesh,
            in_specs=(PartitionSpec("core"),) * 8,
            out_specs=(PartitionSpec("core"),) * (8 + n_outs),
        )
    )

    def _post(o):
        other = jax.lax.ppermute(
            o, "core", [(0, 1), (1, 0), (2, 3), (3, 2), (4, 5), (5, 4), (6, 7), (7, 6)]
        )
        s = o.astype(jnp.float32) + other.astype(jnp.float32)
        idx = jax.lax.axis_index("core")
        return jax.lax.dynamic_slice(s, ((idx % 2) * (T // 2), 0), (T // 2, C))

    post_fn = jax.jit(
        shard_map(
            _post,
            mesh=mesh,
            in_specs=(PartitionSpec("core"),),
            out_specs=PartitionSpec("core"),
        )
    )

    _STATE = dict(
        jax=jax,
        nc=nc,
        in_names=in_names,
        out_names=out_names,
        n_outs=n_outs,
        sharded=sharded,
        zeros_fn=zeros_fn,
        shd=shd,
        statics=statics,
        pre_fn=pre_fn,
        post_fn=post_fn,
    )
    return _STATE


def _run_gathered(st, x, W_attn, W_proj):
    jax = st["jax"]
    import ml_dtypes
    xs = np.ascontiguousarray(x.reshape(8 * (T // 2), C)).astype(
        ml_dtypes.bfloat16
    )
    d_xs = jax.device_put(xs, st["shd"])
    host = (_prep_x8(x) + _prep_w(W_attn, W_proj))
    dev = [jax.device_put(a, st["shd"]) for a in host]
    pre = st["pre_fn"](d_xs, *dev)
    dyn = {"xT": pre[0], "x8": pre[1], "xlo8": pre[2], "w8": pre[3],
           "w2_8": pre[4], "wlo8": pre[5], "wv": pre[6], "wproj": pre[7]}
    args = [dyn[nm] if nm in dyn else st["statics"][nm] for nm in st["in_names"]]
    outs = st["sharded"](*args, *pre[8:])
    po = st["post_fn"](outs[0])
    r = np.asarray(po).reshape(B, T, C)
    return np.ascontiguousarray(r)


def _run_rbks(x, W_attn, W_proj):
    """Fallback: the stock run_bass_kernel_spmd entry point."""
    import ml_dtypes
    bf = ml_dtypes.bfloat16
    nc = _get_nc()
    tabs = _tables_np()
    scale = np.float32(1.0 / np.sqrt(D))
    in_maps = []
    for core in range(8):
        b, hg = core // 2, core % 2
        cs = slice(hg * HPC * D, (hg + 1) * HPC * D)
        wq = W_attn[:, 0 * C:][:, cs] * scale
        wk = W_attn[:, 1 * C:][:, cs]
        wv = W_attn[:, 2 * C:][:, cs]
        w32 = np.concatenate([wq, wk], axis=1) * np.float32(32.0)
        w8n = _to_fp8(w32)
        x8b = _to_fp8(x[b])
        xlob = _to_fp8(16.0 * (x[b] - x8b.astype(np.float32)))
        m = {
            "xT": np.ascontiguousarray(x[b].T).astype(bf),
            "x8": np.ascontiguousarray(x8b.T),
            "xlo8": np.ascontiguousarray(xlob.T),
            "w8": w8n,
            "w2_8": _to_fp8(w32 / 16.0),
            "wlo8": _to_fp8(w32 - w8n.astype(np.float32)),
            "wv": np.ascontiguousarray(wv).astype(bf),
            "wproj": np.ascontiguousarray(W_proj[cs, :]).astype(bf),
        }
        m.update(tabs)
        in_maps.append(m)
    res = run_bass_kernel_spmd(nc, in_maps, core_ids=list(range(8)))
    out = np.empty((B, T, C), dtype=np.float32)
    for b in range(B):
        out[b] = res.results[2 * b]["out"].astype(np.float32) + res.results[
            2 * b + 1
        ]["out"].astype(np.float32)
    return out


def kernel(x, W_attn, W_proj):
    x = np.asarray(x, dtype=np.float32)
    W_attn = np.asarray(W_attn, dtype=np.float32)
    W_proj = np.asarray(W_proj, dtype=np.float32)

    try:
        st = _get_state()
        return _run_gathered(st, x, W_attn, W_proj)
    except Exception:
        return _run_rbks(x, W_attn, W_proj)


if __name__ == "__main__":
    nc = _get_nc()
    from concourse.timeline_sim import TimelineSim
    sim_ns = TimelineSim(nc, trace=False).simulate()
    print(f"timeline-sim: {sim_ns/1e3:.1f} us")
